# revision 1
# baseline (speedup 1.0000x reference)
"""DropBlock (B,C,H,W)=(64,256,64,64), block_size=5 on 8 NeuronCores.

Data-parallel over batch: each core gets 8 batches = 2048 channels.
Per core:
  pass 1: keep = sign(u - gamma) in {-1,+1}; separable 5-tap min-dilation
          (log-trick: 3 DVE min ops per axis) over padded (+1) buffers;
          convert to {0,1} fp8 mask (ACT Copy 0.5*x+0.5) with free
          per-partition count via accum_out; fp8 masks go to DRAM scratch
          except the last RESIDENT blocks which stay in SBUF.
  count:  reduce + partition_all_reduce + 32-byte AllGather over 8 cores +
          local sum, scale = countM / count_ones, broadcast to partitions.
  pass 2: out = (mask8 * scale) * x in one fused scalar_tensor_tensor
          (resident blocks first; x tiles prefetch during pass 1 / the
          collective bubble).
"""

import numpy as np

import concourse.bass_isa as bass_isa
import concourse.mybir as mybir
import concourse.tile as tile
from concourse import bacc, bass_utils

# Problem constants (fixed by the task)
B, C, H, W = 64, 256, 64, 64
BS = 5
HM = WM = 60           # mask resolution H-(BS-1)
N_CORES = 8
B_SH = B // N_CORES    # 8 batches per core
CH = B_SH * C          # 2048 channels per core
P = 128                # partitions
NBLK = CH // P         # 16 channel blocks per core
UF = HM * WM           # 3600 u elems per channel
XF = H * W             # 4096 out elems per channel
HP = H + BS - 1        # 68 (H-padded rows)
MPF = HP * WM          # 4080 flat size of H-padded mask
WP5 = W + BS - 1       # 68 (W-padded cols)
WPF = H * WP5          # 4352 flat size of W-padded buffer
COUNT_M = float(B * C * H * W)
RESIDENT = 5           # last blocks whose fp8 mask stays in SBUF

f32 = mybir.dt.float32
bf16 = mybir.dt.bfloat16
fp8 = mybir.dt.float8e4
AF = mybir.ActivationFunctionType
OP = mybir.AluOpType

TRACE = False
TRACE_KW = {}


def _build_nc(gamma_val: float):
    nc = bacc.Bacc(
        "TRN2", target_bir_lowering=False, debug=False, num_devices=N_CORES
    )

    u_d = nc.dram_tensor("u", [CH, UF], f32, kind="ExternalInput").ap()
    x_d = nc.dram_tensor("x", [CH, XF], f32, kind="ExternalInput").ap()
    g_d = nc.dram_tensor("gamma", [1, 1], f32, kind="ExternalInput").ap()
    o_d = nc.dram_tensor("out", [CH, XF], f32, kind="ExternalOutput").ap()

    with tile.TileContext(nc) as tc:
        with (
            tc.tile_pool(name="fixed", bufs=1) as fixed,
            tc.tile_pool(name="m8_pool", bufs=RESIDENT) as m8_pool,
            tc.tile_pool(name="xpool", bufs=6) as xpool,
            tc.tile_pool(name="m8in", bufs=2) as m8in,
            tc.tile_pool(name="dram", bufs=1, space="DRAM") as dram,
        ):
            mask_dram = dram.tile([CH, XF], fp8, name="mask_dram")
            cc_in = dram.tile([1, 8], f32, name="cc_in")
            cc_out = dram.tile([8, 8], f32, name="cc_out")
            cc_win = dram.tile([1, 8], f32, name="cc_win")
            cc_wout = dram.tile([8, 8], f32, name="cc_wout")

            # warmup collective: absorbs ncfw/descriptor cold-start latency
            # while pass 1 runs, so the real AllGather later is fast
            nc.gpsimd.collective_compute(
                "AllGather",
                OP.bypass,
                replica_groups=[list(range(N_CORES))],
                ins=[cc_win.opt()],
                outs=[cc_wout.opt()],
            )

            gbt = fixed.tile([P, 1], f32, name="gbt")
            nc.gpsimd.memset(gbt[:], -gamma_val)
            # tiny Sign op up front pulls in the ACT table load so the first
            # real compare doesn't pay it
            warm = fixed.tile([P, 1], f32, name="warm")
            nc.scalar.activation(warm[:], gbt[:], AF.Sign, bias=0.0, scale=1.0)

            # persistent padded buffers (manual double-buffer so the one-time
            # pad presets survive across iterations)
            mps, wps = [], []
            for i in range(2):
                mp = fixed.tile([P, MPF], bf16, name=f"mp{i}")
                nc.gpsimd.memset(mp[:, 0:240], 1.0)        # pad rows 0..3
                nc.gpsimd.memset(mp[:, 3840:MPF], 1.0)     # pad rows 64..67
                mps.append(mp)
                wp = fixed.tile([P, WPF], bf16, name=f"wp{i}")
                nc.gpsimd.memset(wp[:], 1.0)               # pad cols stay 1
                wps.append(wp)

            acc = fixed.tile([P, NBLK], f32, name="acc")
            m8_tiles = {}

            with (
                tc.tile_pool(name="upool", bufs=3) as upool,
                tc.tile_pool(name="sh1", bufs=1) as sh1,
                tc.tile_pool(name="sh2", bufs=1) as sh2,
                tc.tile_pool(name="bm_pool", bufs=1) as bm_pool,
            ):
                # ---------------- pass 1 ----------------
                HALF = UF // 2
                for k in range(NBLK):
                    rows = slice(k * P, (k + 1) * P)
                    mp = mps[k % 2]
                    # u in two half-tiles (halves the upool footprint) and
                    # keep = sign(u - gamma) into padded rows 4..63
                    for h in range(2):
                        uh = upool.tile([P, HALF], f32, name="uh")
                        nc.sync.dma_start(
                            uh[:], u_d[rows, h * HALF : (h + 1) * HALF]
                        )
                        nc.scalar.activation(
                            mp[:, 240 + h * HALF : 240 + (h + 1) * HALF],
                            uh[:], AF.Sign, bias=gbt[:, :], scale=1.0,
                        )

                    # H-dilation (min over rows j..j+4), flat shifted views
                    r2b = sh1.tile([P, 3960], bf16, name="r2b", tag="t1")
                    nc.vector.tensor_tensor(
                        r2b[:, 0:3960], mp[:, 0:3960], mp[:, 60:4020], op=OP.min
                    )
                    r4b = sh2.tile([P, 3840], bf16, name="r4b", tag="t2")
                    nc.vector.tensor_tensor(
                        r4b[:, 0:3840], r2b[:, 0:3840], r2b[:, 120:3960],
                        op=OP.min,
                    )
                    wp = wps[k % 2]
                    r4b3 = r4b.rearrange("p (h w) -> p h w", h=H)     # [P,64,60]
                    mp3 = mp.rearrange("p (h w) -> p h w", h=HP)      # [P,68,60]
                    wp3 = wp.rearrange("p (h w) -> p h w", h=H)       # [P,64,68]
                    nc.vector.tensor_tensor(
                        wp3[:, :, 4:64], r4b3[:, :, :], mp3[:, 4:68, :], op=OP.min
                    )

                    # W-dilation (min over cols c..c+4), 3D views skip pad cols
                    q2b = sh1.tile([P, WPF], bf16, name="q2b", tag="t1")
                    q2b3 = q2b.rearrange("p (h w) -> p h w", h=H)
                    nc.vector.tensor_tensor(
                        q2b3[:, :, 0:66], wp3[:, :, 0:66], wp3[:, :, 2:68],
                        op=OP.min,
                    )
                    q4b = sh2.tile([P, WPF], bf16, name="q4b", tag="t2")
                    q4b3x = q4b.rearrange("p (h w) -> p h w", h=H)
                    nc.vector.tensor_tensor(
                        q4b3x[:, :, 0:66], q2b3[:, :, 0:66], q2b3[:, :, 1:67],
                        op=OP.min,
                    )
                    bm = bm_pool.tile([P, XF], bf16, name="bm")
                    q4b3 = q4b.rearrange("p (h w) -> p h w", h=H)     # [P,64,68]
                    bm3 = bm.rearrange("p (h w) -> p h w", h=H)       # [P,64,64]
                    nc.vector.tensor_tensor(
                        bm3[:, :, :], q4b3[:, :, 0:64], wp3[:, :, 4:68], op=OP.min
                    )

                    # {-1,+1} -> {0,1} fp8 + per-partition count (free)
                    m8 = m8_pool.tile([P, XF], fp8, name="m8")
                    nc.scalar.activation(
                        m8[:], bm[:], AF.Copy, bias=0.5, scale=0.5,
                        accum_out=acc[:, k : k + 1],
                    )
                    if k < NBLK - RESIDENT:
                        nc.sync.dma_start(mask_dram[rows, :], m8[:])
                    else:
                        m8_tiles[k] = m8

                # ---------------- global count + scale ----------------
                psum_t = fixed.tile([P, 1], f32, name="psum_t")
                nc.vector.reduce_sum(psum_t[:], acc[:], axis=mybir.AxisListType.X)
                tot_t = fixed.tile([P, 1], f32, name="tot_t")
                nc.gpsimd.partition_all_reduce(
                    tot_t[:], psum_t[:], channels=P,
                    reduce_op=bass_isa.ReduceOp.add,
                )
                # only cc_in[0,0] is meaningful; peers' gathered cols 1..7
                # are never read
                nc.sync.dma_start(cc_in[0:1, 0:1], tot_t[0:1, :])
                nc.gpsimd.collective_compute(
                    "AllGather",
                    OP.bypass,
                    replica_groups=[list(range(N_CORES))],
                    ins=[cc_in.opt()],
                    outs=[cc_out.opt()],
                )
                gath = fixed.tile([1, 8], f32, name="gath")
                nc.sync.dma_start(gath[:], cc_out[:, 0:1])
                tot2 = fixed.tile([1, 1], f32, name="tot2")
                nc.vector.reduce_sum(tot2[:], gath[:], axis=mybir.AxisListType.X)
                rcp = fixed.tile([1, 1], f32, name="rcp")
                nc.vector.reciprocal(rcp[:], tot2[:])
                scl = fixed.tile([1, 1], f32, name="scl")
                nc.vector.tensor_scalar_mul(scl[:], rcp[:], COUNT_M)
                scl_b = fixed.tile([P, 1], f32, name="scl_b")
                nc.gpsimd.partition_broadcast(scl_b[:], scl[:])

            # ---------------- pass 2 (resident-mask blocks first) --------
            order = list(range(NBLK - RESIDENT, NBLK)) + list(
                range(NBLK - RESIDENT)
            )
            with (
                tc.tile_pool(name="opool", bufs=2) as opool,
                tc.tile_pool(name="xpool2", bufs=1) as xpool2,
            ):
                for idx, k in enumerate(order):
                    rows = slice(k * P, (k + 1) * P)
                    xp = xpool2 if idx == 5 else xpool
                    xt = xp.tile([P, XF], f32, name="xt")
                    nc.sync.dma_start(xt[:], x_d[rows, :])
                    if k in m8_tiles:
                        m8s = m8_tiles[k]
                    else:
                        m8s = m8in.tile([P, XF], fp8, name="m8s")
                        nc.sync.dma_start(m8s[:], mask_dram[rows, :])
                    ot = opool.tile([P, XF], f32, name="ot")
                    nc.vector.scalar_tensor_tensor(
                        ot[:], m8s[:], scl_b[:, :], xt[:],
                        op0=OP.mult, op1=OP.mult,
                    )
                    # SWDGE for stores: separate queue hardware from the
                    # HWDGE loads above -> better r/w overlap
                    nc.gpsimd.dma_start(o_d[rows, :], ot[:])

            # keep the ExternalInput gamma tensor referenced (its value is
            # baked into the Sign bias at build time; kernel() re-builds per
            # value); placed last so it stays off the startup DMA queue
            gt = fixed.tile([1, 1], f32, name="gt")
            nc.sync.dma_start(gt[:], g_d[:, :])

    nc.compile()
    return nc


_CACHE = {}


def _get_nc(gamma_val: float):
    key = ("nc", gamma_val)
    if key not in _CACHE:
        _CACHE[key] = _build_nc(gamma_val)
    return _CACHE[key]


def kernel(x, u, gamma):
    x = np.ascontiguousarray(np.asarray(x, dtype=np.float32))
    u = np.ascontiguousarray(np.asarray(u, dtype=np.float32))
    g = np.asarray(gamma, dtype=np.float32).reshape(1, 1)
    nc = _get_nc(float(g[0, 0]))
    in_maps = []
    for i in range(N_CORES):
        xs = x[i * B_SH : (i + 1) * B_SH].reshape(CH, XF)
        us = u[i * B_SH : (i + 1) * B_SH].reshape(CH, UF)
        in_maps.append({"x": xs, "u": us, "gamma": g})
    if "warmed" not in _CACHE:
        # first exec in a process is ~70us slower (cold NEFF/DMA/collective
        # paths); run once untimed so measured runs are steady-state
        bass_utils.run_bass_kernel_spmd(
            nc, in_maps, core_ids=list(range(N_CORES)), trace=False
        )
        _CACHE["warmed"] = True
    res = bass_utils.run_bass_kernel_spmd(
        nc, in_maps, core_ids=list(range(N_CORES)), trace=TRACE, **TRACE_KW
    )
    _CACHE["last_result"] = res
    out = np.concatenate(
        [res.results[i]["out"].reshape(B_SH, C, H, W) for i in range(N_CORES)],
        axis=0,
    )
    return out



# revision 4
# speedup vs baseline: 1.1202x; 1.1202x over previous
"""DropBlock (B,C,H,W)=(64,256,64,64), block_size=5 on 8 NeuronCores.

Data-parallel over batch: each core gets 8 batches = 2048 channels.

Packed-OR dilation: 4 channels are interleaved as fp8 {+-1} bytes inside
int32 words ([row][w-word][4 channel-bytes]).  min-dilation over {-1,+1}
== bitwise OR of the fp8 bytes (only the sign bit differs), so the
separable 5-tap dilation runs as 6 int32 tensor_tensor OR ops per
PACKED block of 512 channels -- 4 mask values per lane-cycle, 2x the
bf16 min path.  W-shifts are whole words (4B-aligned); H-shifts are
whole rows.

Per core:
  pass 1 (4 packed blocks of 512 channels):
    per byte-lane b: DMA u half-tiles, ACT Sign (bias -gamma) writes
    fp8 {+-1} strided into byte-lane b of the packed row buffer;
    6 int32 OR ops (H then W log-tree) -> packed dilated mask pr;
    4 ACT Copies (0.5x+0.5, strided fp8 in) -> per-x-block {0,1} fp8
    m8 tiles (ALL 16 resident in SBUF) with free per-partition counts.
  count: reduce + partition_all_reduce + 32B AllGather over 8 cores +
    local sum, scale = countM / count_ones, broadcast to partitions.
  pass 2: out = (m8 * scale) * x fused scalar_tensor_tensor; x streams
    through a wide pool (space freed by pass-1 scratch), SWDGE stores.
"""

import numpy as np

import concourse.bass_isa as bass_isa
import concourse.mybir as mybir
import concourse.tile as tile
from concourse import bacc, bass_utils

# Problem constants (fixed by the task)
B, C, H, W = 64, 256, 64, 64
BS = 5
HM = WM = 60           # mask resolution H-(BS-1)
N_CORES = 8
B_SH = B // N_CORES    # 8 batches per core
CH = B_SH * C          # 2048 channels per core
P = 128                # partitions
NBLK = CH // P         # 16 x-blocks per core
NPB = NBLK // 4        # 4 packed mask blocks (4 channels/byte-lane each)
UF = HM * WM           # 3600 u elems per channel
XF = H * W             # 4096 out elems per channel
HP = H + BS - 1        # 68 H-padded rows
WP = W + BS - 1        # 68 W-padded word-cols
COUNT_M = float(B * C * H * W)

f32 = mybir.dt.float32
fp8 = mybir.dt.float8e4
i32 = mybir.dt.int32
AF = mybir.ActivationFunctionType
OP = mybir.AluOpType

TRACE = False
TRACE_KW = {}


def _build_nc(gamma_val: float):
    nc = bacc.Bacc(
        "TRN2", target_bir_lowering=False, debug=False, num_devices=N_CORES
    )

    u_d = nc.dram_tensor("u", [CH, UF], f32, kind="ExternalInput").ap()
    x_d = nc.dram_tensor("x", [CH, XF], f32, kind="ExternalInput").ap()
    g_d = nc.dram_tensor("gamma", [1, 1], f32, kind="ExternalInput").ap()
    o_d = nc.dram_tensor("out", [CH, XF], f32, kind="ExternalOutput").ap()

    with tile.TileContext(nc) as tc:
        with (
            tc.tile_pool(name="fixed", bufs=1) as fixed,
            tc.tile_pool(name="m8_pool", bufs=1) as m8_pool,
            tc.tile_pool(name="dram", bufs=1, space="DRAM") as dram,
        ):
            cc_in = dram.tile([1, 8], f32, name="cc_in")
            cc_out = dram.tile([8, 8], f32, name="cc_out")
            cc_win = dram.tile([1, 8], f32, name="cc_win")
            cc_wout = dram.tile([8, 8], f32, name="cc_wout")

            # warmup collective: absorbs ncfw/descriptor cold-start latency
            # while pass 1 runs, so the real AllGather later is fast
            nc.gpsimd.collective_compute(
                "AllGather",
                OP.bypass,
                replica_groups=[list(range(N_CORES))],
                ins=[cc_win.opt()],
                outs=[cc_wout.opt()],
            )

            gbt = fixed.tile([P, 1], f32, name="gbt")
            nc.gpsimd.memset(gbt[:], -gamma_val)
            # tiny Sign op up front pulls in the ACT table load so the first
            # real compare doesn't pay it
            warm = fixed.tile([P, 1], f32, name="warm")
            nc.scalar.activation(warm[:], gbt[:], AF.Sign, bias=0.0, scale=1.0)

            acc = fixed.tile([P, NBLK], f32, name="acc")
            m8_tiles = []
            for k in range(NBLK):
                m8_tiles.append(m8_pool.tile([P, XF], fp8, name=f"m8_{k}"))

            with (
                tc.tile_pool(name="scratch", bufs=1) as scratch,
                tc.tile_pool(name="upool", bufs=3) as upool,
            ):
                # packed row buffers, fp8 bytes, int32-word views.
                # mp: [68 rows x 60 words x 4 bytes]; pad rows 0..3 / 64..67
                # stay +1.0 (byte 0x38) -- OR-neutral.
                mps = []
                for i in range(2):
                    mp = scratch.tile([P, HP * WM * 4], fp8, name=f"mp{i}")
                    nc.gpsimd.memset(mp[:, 0 : 4 * WM * 4], 1.0)
                    nc.gpsimd.memset(mp[:, 64 * WM * 4 : HP * WM * 4], 1.0)
                    mps.append(mp)
                # wp: [64 rows x 68 words x 4 bytes]; pad word-cols 0..3 and
                # 64..67 stay +1.0
                wp = scratch.tile([P, H * WP * 4], fp8, name="wp")
                wpr = wp.rearrange("p (h w) -> p h w", h=H)   # rows of 272 B
                nc.gpsimd.memset(wpr[:, :, 0:16], 1.0)
                nc.gpsimd.memset(wpr[:, :, 256:272], 1.0)
                wp3 = wp.bitcast(i32).rearrange("p (h w) -> p h w", h=H)

                sh1 = scratch.tile([P, H * WP * 4], fp8, name="sh1")
                sh2 = scratch.tile([P, H * WP * 4], fp8, name="sh2")
                sh1f = sh1.bitcast(i32)
                sh2f = sh2.bitcast(i32)
                sh1_3 = sh1f.rearrange("p (h w) -> p h w", h=H)
                sh2_3 = sh2f.rearrange("p (h w) -> p h w", h=H)

                pr = scratch.tile([P, XF * 4], fp8, name="pr")
                pr3 = pr.bitcast(i32).rearrange("p (h w) -> p h w", h=H)
                pr4 = pr.rearrange("p (f b) -> p f b", b=4)

                HALF = UF // 2

                def emit_copies(pb):
                    # {-1,+1} -> {0,1} fp8 per byte-lane + per-partition count
                    for b in range(4):
                        k = 4 * pb + b
                        nc.scalar.activation(
                            m8_tiles[k][:], pr4[:, :, b],
                            AF.Copy, bias=0.5, scale=0.5,
                            accum_out=acc[:, k : k + 1],
                        )

                for pb in range(NPB):
                    mp = mps[pb % 2]
                    mpf = mp.bitcast(i32)                        # [P, 4080]
                    mp3 = mpf.rearrange("p (h w) -> p h w", h=HP)
                    mp4 = mp.rearrange("p (h w b) -> p h w b", h=HP, b=4)
                    # sign(u - gamma) into byte-lane b, mask rows 4..63
                    for b in range(4):
                        k = 4 * pb + b
                        rows = slice(k * P, (k + 1) * P)
                        for h in range(2):
                            uh = upool.tile([P, HALF], f32, name="uh")
                            nc.sync.dma_start(
                                uh[:], u_d[rows, h * HALF : (h + 1) * HALF]
                            )
                            uh3 = uh.rearrange("p (r w) -> p r w", w=WM)
                            nc.scalar.activation(
                                mp4[:, 4 + 30 * h : 34 + 30 * h, :, b],
                                uh3[:, :, :],
                                AF.Sign, bias=gbt[:, :], scale=1.0,
                            )

                    # software-pipelined ACT queue: previous block's pr ->
                    # m8 conversions sit AFTER this block's Signs so the
                    # scalar engine never stalls on the DVE chain
                    if pb > 0:
                        emit_copies(pb - 1)

                    # H-dilation: OR over rows r..r+4 (shifts 1,2,4 rows)
                    nc.vector.tensor_tensor(
                        sh1f[:, 0:3960], mpf[:, 0:3960], mpf[:, 60:4020],
                        op=OP.bitwise_or,
                    )
                    nc.vector.tensor_tensor(
                        sh2f[:, 0:3840], sh1f[:, 0:3840], sh1f[:, 120:3960],
                        op=OP.bitwise_or,
                    )
                    sh2h = sh2f[:, 0:3840].rearrange(
                        "p (h w) -> p h w", w=WM
                    )                                             # [P,64,60]
                    nc.vector.tensor_tensor(
                        wp3[:, :, 4:64], sh2h[:, :, :], mp3[:, 4:68, :],
                        op=OP.bitwise_or,
                    )
                    # W-dilation: OR over word-cols c..c+4 (shifts 2,1,4)
                    nc.vector.tensor_tensor(
                        sh1_3[:, :, 0:66], wp3[:, :, 0:66], wp3[:, :, 2:68],
                        op=OP.bitwise_or,
                    )
                    nc.vector.tensor_tensor(
                        sh2_3[:, :, 0:65], sh1_3[:, :, 0:65], sh1_3[:, :, 1:66],
                        op=OP.bitwise_or,
                    )
                    nc.vector.tensor_tensor(
                        pr3[:, :, :], sh2_3[:, :, 0:64], wp3[:, :, 4:68],
                        op=OP.bitwise_or,
                    )

                emit_copies(NPB - 1)

                # ---------------- global count + scale ----------------
                psum_t = fixed.tile([P, 1], f32, name="psum_t")
                nc.vector.reduce_sum(psum_t[:], acc[:], axis=mybir.AxisListType.X)
                tot_t = fixed.tile([P, 1], f32, name="tot_t")
                nc.gpsimd.partition_all_reduce(
                    tot_t[:], psum_t[:], channels=P,
                    reduce_op=bass_isa.ReduceOp.add,
                )
                # only cc_in[0,0] is meaningful; peers' gathered cols 1..7
                # are never read
                nc.sync.dma_start(cc_in[0:1, 0:1], tot_t[0:1, :])
                nc.gpsimd.collective_compute(
                    "AllGather",
                    OP.bypass,
                    replica_groups=[list(range(N_CORES))],
                    ins=[cc_in.opt()],
                    outs=[cc_out.opt()],
                )
                gath = fixed.tile([1, 8], f32, name="gath")
                nc.sync.dma_start(gath[:], cc_out[:, 0:1])
                tot2 = fixed.tile([1, 1], f32, name="tot2")
                nc.vector.reduce_sum(tot2[:], gath[:], axis=mybir.AxisListType.X)
                rcp = fixed.tile([1, 1], f32, name="rcp")
                nc.vector.reciprocal(rcp[:], tot2[:])
                scl = fixed.tile([1, 1], f32, name="scl")
                nc.vector.tensor_scalar_mul(scl[:], rcp[:], COUNT_M)
                scl_b = fixed.tile([P, 1], f32, name="scl_b")
                nc.gpsimd.partition_broadcast(scl_b[:], scl[:])

            # ---------------- pass 2 ----------------
            with (
                tc.tile_pool(name="xpool", bufs=5) as xpool,
                tc.tile_pool(name="opool", bufs=2) as opool,
            ):
                for k in range(NBLK):
                    rows = slice(k * P, (k + 1) * P)
                    xt = xpool.tile([P, XF], f32, name="xt")
                    nc.sync.dma_start(xt[:], x_d[rows, :])
                    ot = opool.tile([P, XF], f32, name="ot")
                    nc.vector.scalar_tensor_tensor(
                        ot[:], m8_tiles[k][:], scl_b[:, :], xt[:],
                        op0=OP.mult, op1=OP.mult,
                    )
                    # SWDGE for stores: separate queue hardware from the
                    # HWDGE loads above -> better r/w overlap
                    nc.gpsimd.dma_start(o_d[rows, :], ot[:])

            # keep the ExternalInput gamma tensor referenced (its value is
            # baked into the Sign bias at build time; kernel() re-builds per
            # value); placed last so it stays off the startup DMA queue
            gt = fixed.tile([1, 1], f32, name="gt")
            nc.sync.dma_start(gt[:], g_d[:, :])

    nc.compile()
    return nc


_CACHE = {}


def _get_nc(gamma_val: float):
    key = ("nc", gamma_val)
    if key not in _CACHE:
        _CACHE[key] = _build_nc(gamma_val)
    return _CACHE[key]


def kernel(x, u, gamma):
    x = np.ascontiguousarray(np.asarray(x, dtype=np.float32))
    u = np.ascontiguousarray(np.asarray(u, dtype=np.float32))
    g = np.asarray(gamma, dtype=np.float32).reshape(1, 1)
    nc = _get_nc(float(g[0, 0]))
    in_maps = []
    for i in range(N_CORES):
        xs = x[i * B_SH : (i + 1) * B_SH].reshape(CH, XF)
        us = u[i * B_SH : (i + 1) * B_SH].reshape(CH, UF)
        in_maps.append({"x": xs, "u": us, "gamma": g})
    if "warmed" not in _CACHE:
        # first exec in a process is ~70us slower (cold NEFF/DMA/collective
        # paths); run once untimed so measured runs are steady-state
        bass_utils.run_bass_kernel_spmd(
            nc, in_maps, core_ids=list(range(N_CORES)), trace=False
        )
        _CACHE["warmed"] = True
    res = bass_utils.run_bass_kernel_spmd(
        nc, in_maps, core_ids=list(range(N_CORES)), trace=TRACE, **TRACE_KW
    )
    _CACHE["last_result"] = res
    out = np.concatenate(
        [res.results[i]["out"].reshape(B_SH, C, H, W) for i in range(N_CORES)],
        axis=0,
    )
    return out


# revision 11
# speedup vs baseline: 1.1400x; 1.0177x over previous
"""DropBlock (B,C,H,W)=(64,256,64,64), block_size=5 on 8 NeuronCores.

Data-parallel over batch: each core gets 8 batches = 2048 channels.

Packed-OR dilation: 4 channels are interleaved as fp8 {+-1} bytes inside
int32 words ([row][w-word][4 channel-bytes]).  min-dilation over {-1,+1}
== bitwise OR of the fp8 bytes (only the sign bit differs), so the
separable 5-tap dilation runs as 6 int32 tensor_tensor OR ops per
PACKED block of 512 channels -- 4 mask values per lane-cycle, 2x the
bf16 min path.  W-shifts are whole words (4B-aligned); H-shifts are
whole rows.

Per core:
  pass 1 (4 packed blocks of 512 channels):
    per byte-lane b: DMA u half-tiles, ACT Sign (bias -gamma) writes
    fp8 {+-1} strided into byte-lane b of the packed row buffer;
    6 int32 OR ops (H then W log-tree) -> packed dilated mask pr;
    4 ACT Copies (0.5x+0.5, strided fp8 in) -> per-x-block {0,1} fp8
    m8 tiles (ALL 16 resident in SBUF) with free per-partition counts.
  count: reduce + partition_all_reduce + 32B AllGather over 8 cores +
    local sum, scale = countM / count_ones, broadcast to partitions.
  pass 2: out = (m8 * scale) * x fused scalar_tensor_tensor; x streams
    through a wide pool (space freed by pass-1 scratch), SWDGE stores.
"""

import numpy as np

import concourse.bass_isa as bass_isa
import concourse.mybir as mybir
import concourse.tile as tile
from concourse import bacc, bass_utils

# Problem constants (fixed by the task)
B, C, H, W = 64, 256, 64, 64
BS = 5
HM = WM = 60           # mask resolution H-(BS-1)
N_CORES = 8
B_SH = B // N_CORES    # 8 batches per core
CH = B_SH * C          # 2048 channels per core
P = 128                # partitions
NBLK = CH // P         # 16 x-blocks per core
NPB = NBLK // 4        # 4 packed mask blocks (4 channels/byte-lane each)
UF = HM * WM           # 3600 u elems per channel
XF = H * W             # 4096 out elems per channel
HP = H + BS - 1        # 68 H-padded rows
WP = W + BS - 1        # 68 W-padded word-cols
COUNT_M = float(B * C * H * W)

f32 = mybir.dt.float32
fp8 = mybir.dt.float8e4
i32 = mybir.dt.int32
AF = mybir.ActivationFunctionType
OP = mybir.AluOpType

TRACE = False
TRACE_KW = {}


def _build_nc(gamma_val: float):
    nc = bacc.Bacc(
        "TRN2", target_bir_lowering=False, debug=False, num_devices=N_CORES
    )

    u_d = nc.dram_tensor("u", [CH, UF], f32, kind="ExternalInput").ap()
    x_d = nc.dram_tensor("x", [CH, XF], f32, kind="ExternalInput").ap()
    g_d = nc.dram_tensor("gamma", [1, 1], f32, kind="ExternalInput").ap()
    o_d = nc.dram_tensor("out", [CH, XF], f32, kind="ExternalOutput").ap()

    with tile.TileContext(nc) as tc:
        with (
            tc.tile_pool(name="fixed", bufs=1) as fixed,
            tc.tile_pool(name="m8_pool", bufs=1) as m8_pool,
            tc.tile_pool(name="dram", bufs=1, space="DRAM") as dram,
        ):
            cc_in = dram.tile([1, 8], f32, name="cc_in")
            cc_out = dram.tile([8, 8], f32, name="cc_out")
            cc_win = dram.tile([1, 8], f32, name="cc_win")
            cc_wout = dram.tile([8, 8], f32, name="cc_wout")
            cc_win2 = dram.tile([1, 8], f32, name="cc_win2")
            cc_wout2 = dram.tile([8, 8], f32, name="cc_wout2")

            # warmup collective: absorbs ncfw/descriptor cold-start latency
            # while pass 1 runs, so the real AllGather later is fast
            nc.gpsimd.collective_compute(
                "AllGather",
                OP.bypass,
                replica_groups=[list(range(N_CORES))],
                ins=[cc_win.opt()],
                outs=[cc_wout.opt()],
            )

            gbt = fixed.tile([P, 1], f32, name="gbt")
            nc.gpsimd.memset(gbt[:], -gamma_val)
            # tiny Sign op up front pulls in the ACT table load so the first
            # real compare doesn't pay it
            warm = fixed.tile([P, 1], f32, name="warm")
            nc.scalar.activation(warm[:], gbt[:], AF.Sign, bias=0.0, scale=1.0)

            acc = fixed.tile([P, NBLK], f32, name="acc")
            m8_tiles = []
            for k in range(NBLK):
                m8_tiles.append(m8_pool.tile([P, XF], fp8, name=f"m8_{k}"))

            # x-block 0 prefetched during pass 1 so the first pass-2 multiply
            # fires the moment the scale lands
            xpre = fixed.tile([P, XF], f32, name="xpre")
            nc.sync.dma_start(xpre[:], x_d[0:P, :])

            with (
                tc.tile_pool(name="scratch", bufs=1) as scratch,
                tc.tile_pool(name="upool", bufs=2) as upool,
            ):
                # packed row buffers, fp8 bytes, int32-word views.
                # mp: [68 rows x 60 words x 4 bytes]; pad rows 0..3 / 64..67
                # stay +1.0 (byte 0x38) -- OR-neutral.
                mps = []
                for i in range(2):
                    mp = scratch.tile([P, HP * WM * 4], fp8, name=f"mp{i}")
                    nc.gpsimd.memset(mp[:, 0 : 4 * WM * 4], 1.0)
                    nc.gpsimd.memset(mp[:, 64 * WM * 4 : HP * WM * 4], 1.0)
                    mps.append(mp)
                # wp: [64 rows x 68 words x 4 bytes]; pad word-cols 0..3 and
                # 64..67 stay +1.0
                wp = scratch.tile([P, H * WP * 4], fp8, name="wp")
                wpr = wp.rearrange("p (h w) -> p h w", h=H)   # rows of 272 B
                nc.gpsimd.memset(wpr[:, :, 0:16], 1.0)
                nc.gpsimd.memset(wpr[:, :, 256:272], 1.0)
                wp3 = wp.bitcast(i32).rearrange("p (h w) -> p h w", h=H)

                sh1 = scratch.tile([P, H * WP * 4], fp8, name="sh1")
                sh1f = sh1.bitcast(i32)
                sh1_3 = sh1f.rearrange("p (h w) -> p h w", h=H)

                pr = scratch.tile([P, XF * 4], fp8, name="pr")
                pr3 = pr.bitcast(i32).rearrange("p (h w) -> p h w", h=H)
                pr4 = pr.rearrange("p (f b) -> p f b", b=4)

                def emit_copies(pb):
                    # {-1,+1} -> {0,1} fp8 per byte-lane + per-partition count
                    for b in range(4):
                        k = 4 * pb + b
                        nc.scalar.activation(
                            m8_tiles[k][:], pr4[:, :, b],
                            AF.Copy, bias=0.5, scale=0.5,
                            accum_out=acc[:, k : k + 1],
                        )

                for pb in range(NPB):
                    mp = mps[pb % 2]
                    mpf = mp.bitcast(i32)                        # [P, 4080]
                    mp3 = mpf.rearrange("p (h w) -> p h w", h=HP)
                    mp4 = mp.rearrange("p (h w b) -> p h w b", h=HP, b=4)
                    # sign(u - gamma) into byte-lane b, mask rows 4..63
                    for b in range(4):
                        k = 4 * pb + b
                        rows = slice(k * P, (k + 1) * P)
                        ut = upool.tile([P, UF], f32, name="ut")
                        nc.sync.dma_start(ut[:], u_d[rows, :])
                        ut3 = ut.rearrange("p (r w) -> p r w", w=WM)
                        nc.scalar.activation(
                            mp4[:, 4:64, :, b], ut3[:, :, :],
                            AF.Sign, bias=gbt[:, :], scale=1.0,
                        )

                    # software-pipelined ACT queue: previous block's pr ->
                    # m8 conversions sit AFTER this block's Signs so the
                    # scalar engine never stalls on the DVE chain
                    if pb > 0:
                        emit_copies(pb - 1)
                    if pb == NPB - 1:
                        # pipelining warmup: a second AllGather, gated on
                        # block NPB-2's last count cell, keeps the cc stream
                        # hot so the REAL AllGather below doesn't pay a ~40us
                        # cold restart
                        nc.sync.dma_start(
                            cc_win2[0:1, 0:1], acc[0:1, 4 * pb - 1 : 4 * pb]
                        )
                        nc.gpsimd.collective_compute(
                            "AllGather",
                            OP.bypass,
                            replica_groups=[list(range(N_CORES))],
                            ins=[cc_win2.opt()],
                            outs=[cc_wout2.opt()],
                        )

                    # H-dilation: OR over rows r..r+4 (shifts 1,2,4 rows).
                    # Step 2 runs in place on sh1: each output only reads
                    # positions at-or-ahead of itself, and the DVE write-back
                    # lags the reads, so the overlap is safe.
                    nc.vector.tensor_tensor(
                        sh1f[:, 0:3960], mpf[:, 0:3960], mpf[:, 60:4020],
                        op=OP.bitwise_or,
                    )
                    nc.vector.tensor_tensor(
                        sh1f[:, 0:3840], sh1f[:, 0:3840], sh1f[:, 120:3960],
                        op=OP.bitwise_or,
                    )
                    sh1h = sh1f[:, 0:3840].rearrange(
                        "p (h w) -> p h w", w=WM
                    )                                             # [P,64,60]
                    nc.vector.tensor_tensor(
                        wp3[:, :, 4:64], sh1h[:, :, :], mp3[:, 4:68, :],
                        op=OP.bitwise_or,
                    )
                    # W-dilation: OR over word-cols c..c+4 (shifts 2,1,4);
                    # step 2 again in place on sh1.
                    nc.vector.tensor_tensor(
                        sh1_3[:, :, 0:66], wp3[:, :, 0:66], wp3[:, :, 2:68],
                        op=OP.bitwise_or,
                    )
                    nc.vector.tensor_tensor(
                        sh1_3[:, :, 0:64], sh1_3[:, :, 0:64], sh1_3[:, :, 1:65],
                        op=OP.bitwise_or,
                    )
                    nc.vector.tensor_tensor(
                        pr3[:, :, :], sh1_3[:, :, 0:64], wp3[:, :, 4:68],
                        op=OP.bitwise_or,
                    )

                emit_copies(NPB - 1)

                # ---------------- global count + scale ----------------
                psum_t = fixed.tile([P, 1], f32, name="psum_t")
                nc.vector.reduce_sum(psum_t[:], acc[:], axis=mybir.AxisListType.X)
                tot_t = fixed.tile([P, 1], f32, name="tot_t")
                nc.gpsimd.partition_all_reduce(
                    tot_t[:], psum_t[:], channels=P,
                    reduce_op=bass_isa.ReduceOp.add,
                )
                # only cc_in[0,0] is meaningful; peers' gathered cols 1..7
                # are never read
                nc.sync.dma_start(cc_in[0:1, 0:1], tot_t[0:1, :])
                nc.gpsimd.collective_compute(
                    "AllGather",
                    OP.bypass,
                    replica_groups=[list(range(N_CORES))],
                    ins=[cc_in.opt()],
                    outs=[cc_out.opt()],
                )
                gath = fixed.tile([1, 8], f32, name="gath")
                nc.sync.dma_start(gath[:], cc_out[:, 0:1])
                tot2 = fixed.tile([1, 1], f32, name="tot2")
                nc.vector.reduce_sum(tot2[:], gath[:], axis=mybir.AxisListType.X)
                rcp = fixed.tile([1, 1], f32, name="rcp")
                nc.vector.reciprocal(rcp[:], tot2[:])
                scl = fixed.tile([1, 1], f32, name="scl")
                nc.vector.tensor_scalar_mul(scl[:], rcp[:], COUNT_M)
                scl_b = fixed.tile([P, 1], f32, name="scl_b")
                nc.gpsimd.partition_broadcast(scl_b[:], scl[:])

            # ---------------- pass 2 ----------------
            with (
                tc.tile_pool(name="xpool", bufs=5) as xpool,
                tc.tile_pool(name="opool", bufs=2) as opool,
            ):
                for k in range(NBLK):
                    rows = slice(k * P, (k + 1) * P)
                    if k == 0:
                        xt = xpre
                    else:
                        xt = xpool.tile([P, XF], f32, name="xt")
                        nc.sync.dma_start(xt[:], x_d[rows, :])
                    ot = opool.tile([P, XF], f32, name="ot")
                    nc.vector.scalar_tensor_tensor(
                        ot[:], m8_tiles[k][:], scl_b[:, :], xt[:],
                        op0=OP.mult, op1=OP.mult,
                    )
                    # SWDGE for stores: separate queue hardware from the
                    # HWDGE loads above -> better r/w overlap
                    nc.gpsimd.dma_start(o_d[rows, :], ot[:])

            # keep the ExternalInput gamma tensor referenced (its value is
            # baked into the Sign bias at build time; kernel() re-builds per
            # value); placed last so it stays off the startup DMA queue
            gt = fixed.tile([1, 1], f32, name="gt")
            nc.sync.dma_start(gt[:], g_d[:, :])

    nc.compile()
    return nc


_CACHE = {}


def _get_nc(gamma_val: float):
    key = ("nc", gamma_val)
    if key not in _CACHE:
        _CACHE[key] = _build_nc(gamma_val)
    return _CACHE[key]


def kernel(x, u, gamma):
    x = np.ascontiguousarray(np.asarray(x, dtype=np.float32))
    u = np.ascontiguousarray(np.asarray(u, dtype=np.float32))
    g = np.asarray(gamma, dtype=np.float32).reshape(1, 1)
    nc = _get_nc(float(g[0, 0]))
    in_maps = []
    for i in range(N_CORES):
        xs = x[i * B_SH : (i + 1) * B_SH].reshape(CH, XF)
        us = u[i * B_SH : (i + 1) * B_SH].reshape(CH, UF)
        in_maps.append({"x": xs, "u": us, "gamma": g})
    if "warmed" not in _CACHE:
        # first exec in a process is ~70us slower (cold NEFF/DMA/collective
        # paths); run once untimed so measured runs are steady-state
        bass_utils.run_bass_kernel_spmd(
            nc, in_maps, core_ids=list(range(N_CORES)), trace=False
        )
        _CACHE["warmed"] = True
    res = bass_utils.run_bass_kernel_spmd(
        nc, in_maps, core_ids=list(range(N_CORES)), trace=TRACE, **TRACE_KW
    )
    _CACHE["last_result"] = res
    out = np.concatenate(
        [res.results[i]["out"].reshape(B_SH, C, H, W) for i in range(N_CORES)],
        axis=0,
    )
    return out
